# revision 23
# baseline (speedup 1.0000x reference)
"""Trainium2 Bass kernel for nn_CustomPrecision (macro-precision from argmax
confusion matrix).

Math: t = argmax(y_true, 1), p = argmax(y_pred, 1);
cm = onehot(t)^T @ onehot(p) + confusion_matrix (32x32 histogram);
out = mean(diag(cm) / (colsum(cm) + eps)).

Device strategy (8 NeuronCores, data-parallel over N):
  - each core gets 250k rows; 128 partitions x 1953 rows/partition
    (+ a 16-row tail tile), streamed in 25 chunks (growing head to
    shorten pipeline fill, tapered tail to shorten the drain).
  - chunk loads are software-pipelined LOOKAHEAD=2 chunks ahead of
    their compute in emission order, so the Act engine's dma_start is
    not serialized behind its conversion work (engines are in-order).
  - each f32 chunk is converted to fp16 in TRANSPOSED layout
    [p, class, row] (fp16 argmax == f32 argmax up to ~0.1% tie rows;
    measured end-to-end rel-err 1e-5, tolerance is 2e-2).  The
    conversion's strided f32 reads run ~3x below roofline under the
    concurrent DMA write stream (SBUF port contention), so it is split
    DVE:Act = 10:22 classes to balance DVE(conv+tree+eq) against
    Act(conv+dma issue) at the ~8.1us/chunk DMA window.
  - The transposed layout makes every DVE operand packed on the
    innermost (row) axis, so the DVE runs in its 2x_1p mode (2 elem/
    cycle/lane) for both the 5-level pairwise max tree over classes
    and the is_equal mask build (rowmax broadcast is over the OUTER
    class dim, keeping the innermost axis stride-1).
  - 32x32 confusion matrix accumulated on the TensorEngine in PSUM:
    one [K=128]x[32x32] fp16 matmul per 128 rows, reading strided
    single-free-dim slices of the transposed masks (0/1 products are
    exact in fp16; the BIR verifier limits both matmul operands to a
    single free dim).
  - the two input streams load via the two HWDGE queues (SP and Act).
  - each core outputs its LOCAL 32x32 matrix; the 8-matrix sum, the
    confusion_matrix input add, and the 32-element precision reduction
    run on host in kernel() (the gather/unshard step).  A device-side
    AllReduce of the 4KB matrix measured 54us of pure latency plus
    ~12us trigger delay -- a fifth of the whole kernel.
"""

import numpy as np

import concourse.bass as bass
import concourse.mybir as mybir
import concourse.tile as tile
from concourse.bass_utils import run_bass_kernel_spmd

F32 = mybir.dt.float32
F16 = mybir.dt.float16
AX = mybir.AxisListType
OP = mybir.AluOpType

NCORES = 8
N = 2_000_000
C = 32
R = N // NCORES              # 250_000 rows per core
P = 128                      # partitions
RP = R // P                  # 1953 rows per partition (main region)
MAIN = P * RP                # 249_984
TAIL = R - MAIN              # 16 rows
EPS = float(np.finfo(np.float32).eps)

# growing head chunks shorten the pipeline-fill bubble; 96 steady-state.
# Tapered tail shortens the post-DMA drain chain.  (Deeper io bufs=4 +
# half/mask bufs=2 + a longer graduated taper measured WORSE, 243us vs
# 219us: the shallower half/mask pools couple conv to eq completion.)
SIZES = [17, 24, 36, 60, 88] + [96] * 17 + [48, 32, 16]
assert sum(SIZES) == RP
LOOKAHEAD = 3  # chunks of DMA issued ahead of compute emission


def _split_multi_waits(nc, max_waits=1):
    """This container's walrus rejects >1 sync-wait per instruction
    ('Too many sync wait commands').  Move excess waits onto same-engine
    InstNoOp's inserted immediately before the carrying instruction —
    the sequencer blocks on each nop first, so gating is preserved."""
    idx = 0
    for bb in nc.main_func.blocks:
        new_list = []
        for ins in bb.instructions:
            si = ins.sync_info
            if si is not None and si.on_wait and len(si.on_wait) > max_waits:
                waits = list(si.on_wait)
                keep = waits[-max_waits:]
                rest = waits[:-max_waits]
                for i in range(0, len(rest), max_waits):
                    nop = mybir.InstNoOp(
                        name=f"splitw-{idx}",
                        engine=ins.engine,
                        ins=[],
                        outs=[],
                        sync_info=mybir.SyncInfo(
                            on_update=[], on_wait=rest[i : i + max_waits]
                        ),
                    )
                    idx += 1
                    nc.register_instruction(nop, overwrite=True)
                    new_list.append(nop)
                ins.sync_info = mybir.SyncInfo(
                    on_update=list(si.on_update or []), on_wait=keep
                )
            new_list.append(ins)
        bb.instructions = new_list


def _build_program():
    nc = bass.Bass("TRN2", num_devices=NCORES)
    yt = nc.dram_tensor("yt", [R, C], F32, kind="ExternalInput")
    yp = nc.dram_tensor("yp", [R, C], F32, kind="ExternalInput")
    out = nc.dram_tensor("out", [C, C], F32, kind="ExternalOutput")

    # [R, C] -> [128, RP, C] row blocks (contiguous per partition)
    ytv = yt.ap()[0:MAIN, :].rearrange("(p r) c -> p r c", p=P)
    ypv = yp.ap()[0:MAIN, :].rearrange("(p r) c -> p r c", p=P)

    with tile.TileContext(nc) as tc:
        with (
            tc.tile_pool(name="io", bufs=4) as io_pool,
            tc.tile_pool(name="half", bufs=3) as half_pool,
            tc.tile_pool(name="mask", bufs=3) as mask_pool,
            tc.tile_pool(name="tree", bufs=2) as tree_pool,
            tc.tile_pool(name="fin", bufs=1) as fin_pool,
            tc.tile_pool(name="ps", bufs=1, space="PSUM") as ps_pool,
        ):
            cm_ps = ps_pool.tile([C, C], F32)

            def load(xt_src, xp_src, parts, k):
                """Issue the two input DMAs for one chunk.  Emitted ahead
                of the compute for chunk ci-LOOKAHEAD so the Act engine's
                dma_start is not serialized behind its conversion work."""
                k2 = 2 * k
                x = io_pool.tile([parts, k2 * C], F32, tag="x")
                nc.sync.dma_start(
                    x[:, 0 : k * C], xt_src.rearrange("p r c -> p (r c)")
                )
                nc.scalar.dma_start(
                    x[:, k * C : k2 * C], xp_src.rearrange("p r c -> p (r c)")
                )
                return x

            # conversion class-split: the strided f32 reads run ~3x below
            # roofline when overlapped with the DMA write stream (SBUF port
            # contention: ~347ns/class on Act, ~178ns/class on DVE); the
            # measured-worse alternatives: scattered fp16 writes ~10x, and
            # gpsimd's Q7 cast (639ns/class) does not pipeline -- it
            # throttles the whole stream.  Balance DVE(conv+tree+eq) vs
            # Act(conv+dma issue) against the ~8.1us/chunk DMA window.
            CONV_DVE = 10    # classes 0..10 cast on DVE, 10..32 on Act

            def compute(x, parts, k, gps=False):
                """Convert one loaded chunk to transposed fp16 and build
                the one-hot masks.  Returns eqT [parts, C, 2k] fp16 where
                columns 0:k are y_true rows and k:2k are y_pred rows."""
                k2 = 2 * k
                x3 = x[:].rearrange("p (r c) -> p r c", c=C)  # [parts, 2k, C]

                # f32 -> fp16 with transpose ([p, r, c] read order c-fast
                # via strided input AP, packed output).
                xh = half_pool.tile([parts, C, k2], F16, tag="xh")
                x3T = x3.rearrange("p r c -> p c r")
                cs = CONV_DVE
                nc.vector.tensor_copy(xh[:, 0:cs, :], x3T[:, 0:cs, :])
                nc.scalar.copy(xh[:, cs:C, :], x3T[:, cs:C, :])

                # DVE 5-level max tree over the class dim; all operands
                # innermost-packed fp16 -> 2x_1p mode.
                tr = tree_pool.tile([parts, 31, k2], F16, tag="tr")
                t1 = tr[:, 0:16, :]
                t2 = tr[:, 16:24, :]
                t3 = tr[:, 24:28, :]
                t4 = tr[:, 28:30, :]
                tm = tr[:, 30:31, :]
                xhv = xh[:]
                nc.vector.tensor_tensor(t1, xhv[:, 0:16, :], xhv[:, 16:32, :], OP.max)
                nc.vector.tensor_tensor(t2, t1[:, 0:8, :], t1[:, 8:16, :], OP.max)
                nc.vector.tensor_tensor(t3, t2[:, 0:4, :], t2[:, 4:8, :], OP.max)
                nc.vector.tensor_tensor(t4, t3[:, 0:2, :], t3[:, 2:4, :], OP.max)
                nc.vector.tensor_tensor(tm, t4[:, 0:1, :], t4[:, 1:2, :], OP.max)

                eqT = mask_pool.tile([parts, C, k2], F16, tag="eq")
                nc.vector.tensor_tensor(
                    eqT[:], xhv, tm.broadcast_to((parts, C, k2)), OP.is_equal
                )
                return eqT[:]

            offs = []
            acc = 0
            for k in SIZES:
                offs.append(acc)
                acc += k

            # 16-row tail first so its serial chain hides under the main
            # pipeline; its matmul is emitted after the first start=True.
            x_tail = load(
                yt.ap()[MAIN:R, :].rearrange("(p r) c -> p r c", p=TAIL),
                yp.ap()[MAIN:R, :].rearrange("(p r) c -> p r c", p=TAIL),
                TAIL,
                1,
            )
            xs = {}
            for ci in range(min(LOOKAHEAD, len(SIZES))):
                a, k = offs[ci], SIZES[ci]
                xs[ci] = load(ytv[:, a : a + k, :], ypv[:, a : a + k, :], P, k)

            eqT_tail = compute(x_tail, TAIL, 1, gps=False)

            # one [128]x[32x32] matmul per 128 rows; lhsT/rhs are single
            # strided free-dim slices of the transposed masks (walrus
            # requires one free dim on the moving operand).
            nc.tensor.matmul(
                cm_ps[:], lhsT=eqT_tail[:, :, 0], rhs=eqT_tail[:, :, 1],
                start=True, stop=False,
            )
            for ci, k in enumerate(SIZES):
                la = ci + LOOKAHEAD
                if la < len(SIZES):
                    a2, k2_ = offs[la], SIZES[la]
                    xs[la] = load(
                        ytv[:, a2 : a2 + k2_, :], ypv[:, a2 : a2 + k2_, :],
                        P, k2_,
                    )
                eqT = compute(xs.pop(ci), P, k)
                last_chunk = ci == len(SIZES) - 1
                for s in range(k):
                    nc.tensor.matmul(
                        cm_ps[:],
                        lhsT=eqT[:, :, s],
                        rhs=eqT[:, :, k + s],
                        start=False,
                        stop=(last_chunk and s == k - 1),
                    )

            # local 32x32 confusion matrix out; the 8-core sum + precision
            # reduction runs on host (the gather/unshard step) — a device
            # AllReduce of the 4KB matrix measured 54us of pure latency.
            cm_sb = fin_pool.tile([C, C], F32)
            nc.vector.tensor_copy(cm_sb[:], cm_ps[:])
            nc.sync.dma_start(out.ap()[:, :], cm_sb[:])

    _split_multi_waits(nc)
    return nc


_NC_CACHE = None


def kernel(y_true: np.ndarray, y_pred: np.ndarray,
           confusion_matrix: np.ndarray) -> np.ndarray:
    global _NC_CACHE
    if _NC_CACHE is None:
        _NC_CACHE = _build_program()
    nc = _NC_CACHE

    y_true = np.ascontiguousarray(y_true, dtype=np.float32)
    y_pred = np.ascontiguousarray(y_pred, dtype=np.float32)
    cm0 = np.ascontiguousarray(confusion_matrix, dtype=np.float32)

    in_maps = []
    for i in range(NCORES):
        in_maps.append(
            {
                "yt": y_true[i * R : (i + 1) * R],
                "yp": y_pred[i * R : (i + 1) * R],
            }
        )
    res = run_bass_kernel_spmd(nc, in_maps, core_ids=list(range(NCORES)))
    # gather/unshard: sum the 8 partial histograms, then the tiny
    # (32-element) precision reduction.
    cm = cm0.astype(np.float64, copy=True)
    for r in res.results:
        cm += r["out"].astype(np.float64)
    tp = np.diag(cm)
    ppos = cm.sum(axis=0)
    prec = (tp.astype(np.float32)
            / (ppos.astype(np.float32) + np.float32(EPS)))
    val = np.float32(np.mean(prec.astype(np.float32)))
    return np.asarray(val, dtype=np.float32)


if __name__ == "__main__":
    rng = np.random.default_rng(0)
    yt = rng.standard_normal((N, C), dtype=np.float32)
    yp = rng.standard_normal((N, C), dtype=np.float32)
    cm = np.zeros((C, C), np.float32)
    got = kernel(yt, yp, cm)
    t = yt.argmax(1)
    p = yp.argmax(1)
    cmref = np.zeros((C, C), np.float64)
    np.add.at(cmref, (t, p), 1.0)
    tp = np.diag(cmref)
    ppos = cmref.sum(0)
    want = np.mean((tp / (ppos + EPS)).astype(np.float32))
    print("kernel:", got, "numpy:", want, "relerr:", abs(got - want) / abs(want))



# revision 25
# speedup vs baseline: 1.1970x; 1.1970x over previous
"""Trainium2 Bass kernel for nn_CustomPrecision (macro-precision from argmax
confusion matrix).

Math: t = argmax(y_true, 1), p = argmax(y_pred, 1);
cm = onehot(t)^T @ onehot(p) + confusion_matrix (32x32 histogram);
out = mean(diag(cm) / (colsum(cm) + eps)).

Device strategy (8 NeuronCores, data-parallel over N):
  - each core gets 250k rows; 128 partitions x 1953 rows/partition
    (+ a 16-row tail tile), streamed in 25 chunks (growing head to
    shorten pipeline fill, tapered tail to shorten the drain).
  - chunk loads are software-pipelined LOOKAHEAD=2 chunks ahead of
    their compute in emission order, so the Act engine's dma_start is
    not serialized behind its conversion work (engines are in-order).
  - each f32 chunk is converted to fp16 in TRANSPOSED layout
    [p, class, row] (fp16 argmax == f32 argmax up to ~0.1% tie rows;
    measured end-to-end rel-err 1e-5, tolerance is 2e-2).  The
    conversion's strided f32 reads run ~3x below roofline under the
    concurrent DMA write stream (SBUF port contention), so it is split
    DVE:Act = 10:22 classes to balance DVE(conv+tree+eq) against
    Act(conv+dma issue) at the ~8.1us/chunk DMA window.
  - The transposed layout makes every DVE operand packed on the
    innermost (row) axis, so the DVE runs in its 2x_1p mode (2 elem/
    cycle/lane) for both the 5-level pairwise max tree over classes
    and the is_equal mask build (rowmax broadcast is over the OUTER
    class dim, keeping the innermost axis stride-1).
  - 32x32 confusion matrix accumulated on the TensorEngine in PSUM:
    one [K=128]x[32x32] fp16 matmul per 128 rows, reading strided
    single-free-dim slices of the transposed masks (0/1 products are
    exact in fp16; the BIR verifier limits both matmul operands to a
    single free dim).
  - the two input streams load via the two HWDGE queues (SP and Act).
  - each core outputs its LOCAL 32x32 matrix; the 8-matrix sum, the
    confusion_matrix input add, and the 32-element precision reduction
    run on host in kernel() (the gather/unshard step).  A device-side
    AllReduce of the 4KB matrix measured 54us of pure latency plus
    ~12us trigger delay -- a fifth of the whole kernel.
"""

import numpy as np

import concourse.bass as bass
import concourse.mybir as mybir
import concourse.tile as tile
from concourse.bass_utils import run_bass_kernel_spmd

F32 = mybir.dt.float32
F16 = mybir.dt.float16
AX = mybir.AxisListType
OP = mybir.AluOpType

NCORES = 8
N = 2_000_000
C = 32
R = N // NCORES              # 250_000 rows per core
P = 128                      # partitions
RP = R // P                  # 1953 rows per partition (main region)
MAIN = P * RP                # 249_984
TAIL = R - MAIN              # 16 rows
EPS = float(np.finfo(np.float32).eps)

# growing head chunks shorten the pipeline-fill bubble; 96 steady-state.
# Tapered tail shortens the post-DMA drain chain.  (Deeper io bufs=4 +
# half/mask bufs=2 + a longer graduated taper measured WORSE, 243us vs
# 219us: the shallower half/mask pools couple conv to eq completion.)
# Gentle long taper: the post-stream drain equals the compute backlog of
# the in-flight chunks, so the last ~6 chunks shrink gradually -- the
# backlog at stream end is then made of small chunks (~1 chunk-equivalent
# of work) instead of two full 96-row chunks.
SIZES = [17, 24, 36, 60, 88] + [96] * 14 + [88, 80, 72, 64, 48, 32]
assert sum(SIZES) == RP
LOOKAHEAD = 2  # chunks of DMA issued ahead of compute emission


def _split_multi_waits(nc, max_waits=1):
    """This container's walrus rejects >1 sync-wait per instruction
    ('Too many sync wait commands').  Move excess waits onto same-engine
    InstNoOp's inserted immediately before the carrying instruction —
    the sequencer blocks on each nop first, so gating is preserved."""
    idx = 0
    for bb in nc.main_func.blocks:
        new_list = []
        for ins in bb.instructions:
            si = ins.sync_info
            if si is not None and si.on_wait and len(si.on_wait) > max_waits:
                waits = list(si.on_wait)
                keep = waits[-max_waits:]
                rest = waits[:-max_waits]
                for i in range(0, len(rest), max_waits):
                    nop = mybir.InstNoOp(
                        name=f"splitw-{idx}",
                        engine=ins.engine,
                        ins=[],
                        outs=[],
                        sync_info=mybir.SyncInfo(
                            on_update=[], on_wait=rest[i : i + max_waits]
                        ),
                    )
                    idx += 1
                    nc.register_instruction(nop, overwrite=True)
                    new_list.append(nop)
                ins.sync_info = mybir.SyncInfo(
                    on_update=list(si.on_update or []), on_wait=keep
                )
            new_list.append(ins)
        bb.instructions = new_list


def _build_program():
    nc = bass.Bass("TRN2", num_devices=NCORES)
    yt = nc.dram_tensor("yt", [R, C], F32, kind="ExternalInput")
    yp = nc.dram_tensor("yp", [R, C], F32, kind="ExternalInput")
    out = nc.dram_tensor("out", [C, C], F32, kind="ExternalOutput")

    # [R, C] -> [128, RP, C] row blocks (contiguous per partition)
    ytv = yt.ap()[0:MAIN, :].rearrange("(p r) c -> p r c", p=P)
    ypv = yp.ap()[0:MAIN, :].rearrange("(p r) c -> p r c", p=P)

    with tile.TileContext(nc) as tc:
        with (
            tc.tile_pool(name="io", bufs=3) as io_pool,
            tc.tile_pool(name="half", bufs=3) as half_pool,
            tc.tile_pool(name="mask", bufs=3) as mask_pool,
            tc.tile_pool(name="tree", bufs=2) as tree_pool,
            tc.tile_pool(name="fin", bufs=1) as fin_pool,
            tc.tile_pool(name="ps", bufs=1, space="PSUM") as ps_pool,
        ):
            cm_ps = ps_pool.tile([C, C], F32)

            def load(xt_src, xp_src, parts, k):
                """Issue the two input DMAs for one chunk.  Emitted ahead
                of the compute for chunk ci-LOOKAHEAD so the Act engine's
                dma_start is not serialized behind its conversion work."""
                k2 = 2 * k
                x = io_pool.tile([parts, k2 * C], F32, tag="x")
                nc.sync.dma_start(
                    x[:, 0 : k * C], xt_src.rearrange("p r c -> p (r c)")
                )
                nc.scalar.dma_start(
                    x[:, k * C : k2 * C], xp_src.rearrange("p r c -> p (r c)")
                )
                return x

            # conversion class-split: the strided f32 reads run ~3x below
            # roofline when overlapped with the DMA write stream (SBUF port
            # contention: ~347ns/class on Act, ~178ns/class on DVE); the
            # measured-worse alternatives: scattered fp16 writes ~10x, and
            # gpsimd's Q7 cast (639ns/class) does not pipeline -- it
            # throttles the whole stream.  Balance DVE(conv+tree+eq) vs
            # Act(conv+dma issue) against the ~8.1us/chunk DMA window.
            CONV_DVE = 10    # classes 0..10 cast on DVE, 10..32 on Act

            def compute(x, parts, k, gps=False):
                """Convert one loaded chunk to transposed fp16 and build
                the one-hot masks.  Returns eqT [parts, C, 2k] fp16 where
                columns 0:k are y_true rows and k:2k are y_pred rows."""
                k2 = 2 * k
                x3 = x[:].rearrange("p (r c) -> p r c", c=C)  # [parts, 2k, C]

                # f32 -> fp16 with transpose ([p, r, c] read order c-fast
                # via strided input AP, packed output).
                xh = half_pool.tile([parts, C, k2], F16, tag="xh")
                x3T = x3.rearrange("p r c -> p c r")
                cs = CONV_DVE
                nc.vector.tensor_copy(xh[:, 0:cs, :], x3T[:, 0:cs, :])
                nc.scalar.copy(xh[:, cs:C, :], x3T[:, cs:C, :])

                # DVE 5-level max tree over the class dim; all operands
                # innermost-packed fp16 -> 2x_1p mode.
                tr = tree_pool.tile([parts, 31, k2], F16, tag="tr")
                t1 = tr[:, 0:16, :]
                t2 = tr[:, 16:24, :]
                t3 = tr[:, 24:28, :]
                t4 = tr[:, 28:30, :]
                tm = tr[:, 30:31, :]
                xhv = xh[:]
                nc.vector.tensor_tensor(t1, xhv[:, 0:16, :], xhv[:, 16:32, :], OP.max)
                nc.vector.tensor_tensor(t2, t1[:, 0:8, :], t1[:, 8:16, :], OP.max)
                nc.vector.tensor_tensor(t3, t2[:, 0:4, :], t2[:, 4:8, :], OP.max)
                nc.vector.tensor_tensor(t4, t3[:, 0:2, :], t3[:, 2:4, :], OP.max)
                nc.vector.tensor_tensor(tm, t4[:, 0:1, :], t4[:, 1:2, :], OP.max)

                eqT = mask_pool.tile([parts, C, k2], F16, tag="eq")
                nc.vector.tensor_tensor(
                    eqT[:], xhv, tm.broadcast_to((parts, C, k2)), OP.is_equal
                )
                return eqT[:]

            offs = []
            acc = 0
            for k in SIZES:
                offs.append(acc)
                acc += k

            # 16-row tail first so its serial chain hides under the main
            # pipeline; its matmul is emitted after the first start=True.
            x_tail = load(
                yt.ap()[MAIN:R, :].rearrange("(p r) c -> p r c", p=TAIL),
                yp.ap()[MAIN:R, :].rearrange("(p r) c -> p r c", p=TAIL),
                TAIL,
                1,
            )
            xs = {}
            for ci in range(min(LOOKAHEAD, len(SIZES))):
                a, k = offs[ci], SIZES[ci]
                xs[ci] = load(ytv[:, a : a + k, :], ypv[:, a : a + k, :], P, k)

            eqT_tail = compute(x_tail, TAIL, 1, gps=False)

            # one [128]x[32x32] matmul per 128 rows; lhsT/rhs are single
            # strided free-dim slices of the transposed masks (walrus
            # requires one free dim on the moving operand).
            nc.tensor.matmul(
                cm_ps[:], lhsT=eqT_tail[:, :, 0], rhs=eqT_tail[:, :, 1],
                start=True, stop=False,
            )
            for ci, k in enumerate(SIZES):
                la = ci + LOOKAHEAD
                if la < len(SIZES):
                    a2, k2_ = offs[la], SIZES[la]
                    xs[la] = load(
                        ytv[:, a2 : a2 + k2_, :], ypv[:, a2 : a2 + k2_, :],
                        P, k2_,
                    )
                eqT = compute(xs.pop(ci), P, k)
                last_chunk = ci == len(SIZES) - 1
                for s in range(k):
                    nc.tensor.matmul(
                        cm_ps[:],
                        lhsT=eqT[:, :, s],
                        rhs=eqT[:, :, k + s],
                        start=False,
                        stop=(last_chunk and s == k - 1),
                    )

            # local 32x32 confusion matrix out; the 8-core sum + precision
            # reduction runs on host (the gather/unshard step) — a device
            # AllReduce of the 4KB matrix measured 54us of pure latency.
            cm_sb = fin_pool.tile([C, C], F32)
            nc.vector.tensor_copy(cm_sb[:], cm_ps[:])
            nc.sync.dma_start(out.ap()[:, :], cm_sb[:])

    _split_multi_waits(nc)
    return nc


_NC_CACHE = None


def kernel(y_true: np.ndarray, y_pred: np.ndarray,
           confusion_matrix: np.ndarray) -> np.ndarray:
    global _NC_CACHE
    if _NC_CACHE is None:
        _NC_CACHE = _build_program()
    nc = _NC_CACHE

    y_true = np.ascontiguousarray(y_true, dtype=np.float32)
    y_pred = np.ascontiguousarray(y_pred, dtype=np.float32)
    cm0 = np.ascontiguousarray(confusion_matrix, dtype=np.float32)

    in_maps = []
    for i in range(NCORES):
        in_maps.append(
            {
                "yt": y_true[i * R : (i + 1) * R],
                "yp": y_pred[i * R : (i + 1) * R],
            }
        )
    res = run_bass_kernel_spmd(nc, in_maps, core_ids=list(range(NCORES)))
    # gather/unshard: sum the 8 partial histograms, then the tiny
    # (32-element) precision reduction.
    cm = cm0.astype(np.float64, copy=True)
    for r in res.results:
        cm += r["out"].astype(np.float64)
    tp = np.diag(cm)
    ppos = cm.sum(axis=0)
    prec = (tp.astype(np.float32)
            / (ppos.astype(np.float32) + np.float32(EPS)))
    val = np.float32(np.mean(prec.astype(np.float32)))
    return np.asarray(val, dtype=np.float32)


if __name__ == "__main__":
    rng = np.random.default_rng(0)
    yt = rng.standard_normal((N, C), dtype=np.float32)
    yp = rng.standard_normal((N, C), dtype=np.float32)
    cm = np.zeros((C, C), np.float32)
    got = kernel(yt, yp, cm)
    t = yt.argmax(1)
    p = yp.argmax(1)
    cmref = np.zeros((C, C), np.float64)
    np.add.at(cmref, (t, p), 1.0)
    tp = np.diag(cmref)
    ppos = cmref.sum(0)
    want = np.mean((tp / (ppos + EPS)).astype(np.float32))
    print("kernel:", got, "numpy:", want, "relerr:", abs(got - want) / abs(want))

